# revision 4
# baseline (speedup 1.0000x reference)
"""MoE layer (8 experts, top-2) on 8 TRN2 NeuronCores, expert-parallel.

Strategy (dense V1):
  - Core m owns expert m (w1[m], w2[m], b1[m], b2[m]).
  - hidden_states (as X^T [H, T]) is replicated to all cores.
  - Each core computes the router (fp32 matmul on PE) for all T tokens and
    derives ITS OWN expert's combine weight per token purely elementwise:
        w_e(t) = exp(l_e - m1) / (1 + exp(m2 - m1))  if l_e >= m2 else 0
    where m1/m2 are the top-2 logit values. This equals softmax-top2-renorm.
  - Dense FFN in bf16: h1 = gelu(x @ w1 + b1) [F, T-chunk layout],
    y^T = (h1^T @ w2) with tokens on partitions, then  out = w * (y + b2).
  - Each core writes a weighted partial [T, H]; host sums 8 partials.
"""

from contextlib import ExitStack

import ml_dtypes
import numpy as np

P = 128
B, S, H, F, E = 2, 2048, 1024, 4096, 8
T = B * S            # 4096 tokens
KH = H // P          # 8   k-subtiles over H
KF = F // P          # 32  k-subtiles over F
NCHUNK = 512         # token chunk for the FFN
NCH = T // NCHUNK    # 8
TT = T // P          # 32 token tiles (router / output)
CT = NCHUNK // P     # 4 token tiles per chunk
NHALF = H // 512     # 2 halves of H for matmul2 free dim

_CACHE = {}


def _build_nc():
    import concourse.mybir as mybir
    import concourse.tile as tile
    from concourse import bacc

    dt = mybir.dt
    AF = mybir.ActivationFunctionType
    ALU = mybir.AluOpType
    AX = mybir.AxisListType

    nc = bacc.Bacc(
        "TRN2", target_bir_lowering=False, debug=False, num_devices=E)

    xt32 = nc.declare_dram_parameter("xt32", [H, T], dt.float32, isOutput=False)
    xtb = nc.declare_dram_parameter("xtb", [H, T], dt.bfloat16, isOutput=False)
    rw = nc.declare_dram_parameter("rw", [H, E], dt.float32, isOutput=False)
    rbb = nc.declare_dram_parameter("rbb", [P, E], dt.float32, isOutput=False)
    selb = nc.declare_dram_parameter("selb", [P, E], dt.float32, isOutput=False)
    w1d = nc.declare_dram_parameter("w1d", [H, F], dt.bfloat16, isOutput=False)
    w2d = nc.declare_dram_parameter("w2d", [F, H], dt.bfloat16, isOutput=False)
    b1d = nc.declare_dram_parameter("b1d", [P, KF], dt.float32, isOutput=False)
    b2b = nc.declare_dram_parameter("b2b", [P, H], dt.float32, isOutput=False)
    part = nc.declare_dram_parameter("part", [T, H], dt.float32, isOutput=True)

    xt32_r = xt32.rearrange("(k p) t -> p k t", p=P)
    xtb_r = xtb.rearrange("(k p) t -> p k t", p=P)
    rw_r = rw.rearrange("(k p) e -> p k e", p=P)
    w1_r = w1d.rearrange("(k p) f -> p k f", p=P)
    w2_r = w2d.rearrange("(k p) h -> p k h", p=P)

    with ExitStack() as ctx:
        tc = ctx.enter_context(tile.TileContext(nc))
        const = ctx.enter_context(tc.tile_pool(name="const", bufs=1))
        xrpool = ctx.enter_context(tc.tile_pool(name="xr", bufs=2))
        rpool = ctx.enter_context(tc.tile_pool(name="rtmp", bufs=3))
        rpsum = ctx.enter_context(tc.tile_pool(name="rpsum", bufs=2, space="PSUM"))
        xpool = ctx.enter_context(tc.tile_pool(name="xc", bufs=2))
        h1pool = ctx.enter_context(tc.tile_pool(name="h1", bufs=1))
        p1pool = ctx.enter_context(tc.tile_pool(name="p1", bufs=2, space="PSUM"))
        p2pool = ctx.enter_context(tc.tile_pool(name="p2", bufs=2, space="PSUM"))
        opool = ctx.enter_context(tc.tile_pool(name="ob", bufs=4))

        # Resident weights / constants.
        w1_s = const.tile([P, KH, F], dt.bfloat16)
        nc.sync.dma_start(w1_s[:], w1_r)
        w2_s = const.tile([P, KF, H], dt.bfloat16)
        nc.sync.dma_start(w2_s[:], w2_r)
        rw_s = const.tile([P, KH, E], dt.float32)
        nc.sync.dma_start(rw_s[:], rw_r)
        rbb_s = const.tile([P, E], dt.float32)
        nc.sync.dma_start(rbb_s[:], rbb[:])
        selb_s = const.tile([P, E], dt.float32)
        nc.sync.dma_start(selb_s[:], selb[:])
        b1_s = const.tile([P, KF], dt.float32)
        nc.sync.dma_start(b1_s[:], b1d[:])
        b2b_s = const.tile([P, H], dt.float32)
        nc.sync.dma_start(b2b_s[:], b2b[:])
        wmat = const.tile([P, TT], dt.float32)

        # ---- Router: combine weight of MY expert for every token ----
        for tt in range(TT):
            xt_t = xrpool.tile([P, KH, P], dt.float32)
            nc.sync.dma_start(xt_t[:], xt32_r[:, :, tt * P:(tt + 1) * P])
            lg = rpsum.tile([P, E], dt.float32)
            for k in range(KH):
                nc.tensor.matmul(
                    lg[:], xt_t[:, k], rw_s[:, k],
                    start=(k == 0), stop=(k == KH - 1),
                )
            l = rpool.tile([P, E], dt.float32)
            nc.vector.tensor_tensor(l[:], lg[:], rbb_s[:], ALU.add)
            m1 = rpool.tile([P, 1], dt.float32)
            nc.vector.reduce_max(m1[:], l[:], axis=AX.X)
            nm1 = rpool.tile([P, 1], dt.float32)
            nc.vector.tensor_scalar_mul(nm1[:], m1[:], -1.0)
            ismax = rpool.tile([P, E], dt.float32)
            nc.vector.tensor_tensor(
                ismax[:], l[:], m1[:].to_broadcast((P, E)), ALU.is_equal)
            pen = rpool.tile([P, E], dt.float32)
            nc.vector.tensor_scalar_mul(pen[:], ismax[:], 1e30)
            lmask = rpool.tile([P, E], dt.float32)
            nc.vector.tensor_tensor(lmask[:], l[:], pen[:], ALU.subtract)
            m2 = rpool.tile([P, 1], dt.float32)
            nc.vector.reduce_max(m2[:], lmask[:], axis=AX.X)
            lsel = rpool.tile([P, E], dt.float32)
            nc.vector.tensor_tensor(lsel[:], l[:], selb_s[:], ALU.mult)
            lmine = rpool.tile([P, 1], dt.float32)
            nc.vector.reduce_sum(lmine[:], lsel[:], axis=AX.X)
            ge = rpool.tile([P, 1], dt.float32)
            nc.vector.tensor_tensor(ge[:], lmine[:], m2[:], ALU.is_ge)
            e1 = rpool.tile([P, 1], dt.float32)
            nc.scalar.activation(e1[:], lmine[:], AF.Exp, bias=nm1[:])
            e2 = rpool.tile([P, 1], dt.float32)
            nc.scalar.activation(e2[:], m2[:], AF.Exp, bias=nm1[:])
            den = rpool.tile([P, 1], dt.float32)
            nc.vector.tensor_scalar_add(den[:], e2[:], 1.0)
            rec = rpool.tile([P, 1], dt.float32)
            nc.vector.reciprocal(rec[:], den[:])
            wnum = rpool.tile([P, 1], dt.float32)
            nc.vector.tensor_tensor(wnum[:], e1[:], ge[:], ALU.mult)
            nc.vector.tensor_tensor(wmat[:, tt:tt + 1], wnum[:], rec[:], ALU.mult)

        # ---- Expert FFN (dense over all tokens), weighted partial out ----
        for ch in range(NCH):
            t0 = ch * NCHUNK
            xc = xpool.tile([P, KH, NCHUNK], dt.bfloat16)
            nc.sync.dma_start(xc[:], xtb_r[:, :, t0:t0 + NCHUNK])
            h1 = h1pool.tile([P, KF, NCHUNK], dt.bfloat16)
            for f in range(KF):
                ps1 = p1pool.tile([P, NCHUNK], dt.float32)
                for k in range(KH):
                    nc.tensor.matmul(
                        ps1[:], w1_s[:, k, f * P:(f + 1) * P], xc[:, k],
                        start=(k == 0), stop=(k == KH - 1),
                    )
                nc.scalar.activation(h1[:, f], ps1[:], AF.Gelu, bias=b1_s[:, f:f + 1])
            for ct in range(CT):
                gt = ch * CT + ct
                for hh in range(NHALF):
                    ps2 = p2pool.tile([P, 512], dt.float32)
                    for k in range(KF):
                        nc.tensor.matmul(
                            ps2[:], h1[:, k, ct * P:(ct + 1) * P],
                            w2_s[:, k, hh * 512:(hh + 1) * 512],
                            start=(k == 0), stop=(k == KF - 1),
                        )
                    ob = opool.tile([P, 512], dt.float32)
                    nc.vector.tensor_tensor(
                        ob[:], ps2[:], b2b_s[:, hh * 512:(hh + 1) * 512], ALU.add)
                    nc.vector.tensor_scalar_mul(ob[:], ob[:], wmat[:, gt:gt + 1])
                    nc.sync.dma_start(
                        part[gt * P:(gt + 1) * P, hh * 512:(hh + 1) * 512], ob[:])
    return nc


def _get_nc():
    if "nc" not in _CACHE:
        nc = _build_nc()
        nc.finalize()
        _CACHE["nc"] = nc
    return _CACHE["nc"]


def make_in_maps(hidden_states, router_w, router_b, w1, b1, w2, b2):
    bf16 = ml_dtypes.bfloat16
    x = np.ascontiguousarray(
        np.asarray(hidden_states, dtype=np.float32).reshape(T, H).T)  # [H, T]
    xtb = x.astype(bf16)
    rw = np.ascontiguousarray(np.asarray(router_w, dtype=np.float32))
    rbb = np.ascontiguousarray(
        np.broadcast_to(np.asarray(router_b, dtype=np.float32), (P, E)))
    w1 = np.asarray(w1, dtype=np.float32)
    w2 = np.asarray(w2, dtype=np.float32)
    b1 = np.asarray(b1, dtype=np.float32)
    b2 = np.asarray(b2, dtype=np.float32)
    in_maps = []
    for m in range(E):
        sel = np.zeros((P, E), dtype=np.float32)
        sel[:, m] = 1.0
        in_maps.append({
            "xt32": x,
            "xtb": xtb,
            "rw": rw,
            "rbb": rbb,
            "selb": sel,
            "w1d": np.ascontiguousarray(w1[m].astype(bf16)),
            "w2d": np.ascontiguousarray(w2[m].astype(bf16)),
            "b1d": np.ascontiguousarray(b1[m].reshape(KF, P).T),
            "b2b": np.ascontiguousarray(np.broadcast_to(b2[m], (P, H))),
        })
    return in_maps


def run_device(in_maps):
    from concourse.bass_utils import run_bass_kernel_spmd

    nc = _get_nc()
    res = run_bass_kernel_spmd(nc, in_maps, core_ids=list(range(E)))
    return res.results


def kernel(hidden_states, router_w, router_b, w1, b1, w2, b2):
    in_maps = make_in_maps(hidden_states, router_w, router_b, w1, b1, w2, b2)
    results = run_device(in_maps)
    acc = np.zeros((T, H), dtype=np.float32)
    for m in range(E):
        acc += np.asarray(results[m]["part"], dtype=np.float32)
    return acc.reshape(B, S, H)


# revision 12
# speedup vs baseline: 38829.4101x; 38829.4101x over previous
"""MoE layer (8 experts, top-2) on 8 TRN2 NeuronCores, expert-parallel.

Strategy (sparse dispatch, per the sharding hint):
  - Core m owns expert m (w1[m], w2[m], b1[m], b2[m]).
  - Host computes top-2 expert ids per token (fp32 router, dispatch only)
    and "all-to-all"s: each core receives only the tokens routed to its
    expert, gathered as X_c^T [H, C] (C = max expert load, rounded to 128).
  - On device, each core re-runs the router (fp32 matmul on PE) over its
    gathered tokens and derives ITS OWN expert's combine weight per token
    purely elementwise:
        w_e(t) = exp(l_e - m1) / (1 + exp(m2 - m1))  if l_e >= m2 else 0
    (equals softmax-top2-renormalize of the reference).
  - FFN in bf16 (f32 PSUM accumulate): h1 = gelu(x @ w1 + b1) in [F, C]
    layout; y = (h1^T @ w2 + b2) * w with tokens on partitions -> yc [C, H].
  - Host scatter-adds each core's weighted outputs back to token order.
"""

from contextlib import ExitStack

import ml_dtypes
import numpy as np

P = 128
B, S, H, F, E = 2, 2048, 1024, 4096, 8
T = B * S            # 4096 tokens
KH = H // P          # 8   k-subtiles over H
KF = F // P          # 32  k-subtiles over F

_CACHE = {}


def _chunks(C):
    out = []
    t0 = 0
    while t0 < C:
        size = min(512, C - t0)
        out.append((t0, size))
        t0 += size
    return out


def _build_nc(C):
    import concourse.mybir as mybir
    import concourse.tile as tile
    from concourse import bacc

    dt = mybir.dt
    AF = mybir.ActivationFunctionType
    ALU = mybir.AluOpType
    AX = mybir.AxisListType

    TTc = C // P  # token tiles

    nc = bacc.Bacc(
        "TRN2", target_bir_lowering=False, debug=False, num_devices=E)

    xct32 = nc.declare_dram_parameter("xct32", [H, C], dt.float32, isOutput=False)
    xctb = nc.declare_dram_parameter("xctb", [H, C], dt.bfloat16, isOutput=False)
    rw = nc.declare_dram_parameter("rw", [H, E], dt.float32, isOutput=False)
    rbb = nc.declare_dram_parameter("rbb", [P, E], dt.float32, isOutput=False)
    selb = nc.declare_dram_parameter("selb", [P, E], dt.float32, isOutput=False)
    w1d = nc.declare_dram_parameter("w1d", [H, F], dt.bfloat16, isOutput=False)
    w2d = nc.declare_dram_parameter("w2d", [F, H], dt.bfloat16, isOutput=False)
    b1d = nc.declare_dram_parameter("b1d", [P, KF], dt.float32, isOutput=False)
    b2b = nc.declare_dram_parameter("b2b", [P, H], dt.float32, isOutput=False)
    yc = nc.declare_dram_parameter("yc", [C, H], dt.float32, isOutput=True)

    xct32_r = xct32.rearrange("(k p) t -> p k t", p=P)
    xctb_r = xctb.rearrange("(k p) t -> p k t", p=P)
    rw_r = rw.rearrange("(k p) e -> p k e", p=P)
    w1_r = w1d.rearrange("(k p) f -> p k f", p=P)
    w2_r = w2d.rearrange("(k p) h -> p k h", p=P)

    with ExitStack() as ctx:
        tc = ctx.enter_context(tile.TileContext(nc))
        const = ctx.enter_context(tc.tile_pool(name="const", bufs=1))
        xrpool = ctx.enter_context(tc.tile_pool(name="xr", bufs=2))
        rpool = ctx.enter_context(tc.tile_pool(name="rtmp", bufs=3))
        rpsum = ctx.enter_context(tc.tile_pool(name="rpsum", bufs=2, space="PSUM"))
        xpool = ctx.enter_context(tc.tile_pool(name="xc", bufs=2))
        h1pool = ctx.enter_context(tc.tile_pool(name="h1", bufs=1))
        p1pool = ctx.enter_context(tc.tile_pool(name="p1", bufs=2, space="PSUM"))
        p2pool = ctx.enter_context(tc.tile_pool(name="p2", bufs=3, space="PSUM"))
        opool = ctx.enter_context(tc.tile_pool(name="ob", bufs=6))

        # Small constants first so nothing queues behind the weight stacks.
        rw_s = const.tile([P, KH, E], dt.float32)
        nc.sync.dma_start(rw_s[:], rw_r)
        rbb_s = const.tile([P, E], dt.float32)
        nc.sync.dma_start(rbb_s[:], rbb[:])
        selb_s = const.tile([P, E], dt.float32)
        nc.sync.dma_start(selb_s[:], selb[:])
        b1_s = const.tile([P, KF], dt.float32)
        nc.sync.dma_start(b1_s[:], b1d[:])
        b2b_s = const.tile([P, H], dt.float32)
        nc.sync.dma_start(b2b_s[:], b2b[:])
        wmat = const.tile([P, TTc], dt.float32)

        chunks = _chunks(C)

        def load_xc(t0, csz):
            xc = xpool.tile([P, KH, 512], dt.bfloat16, name="xc")[:, :, :csz]
            for k in range(KH):
                nc.sync.dma_start(xc[:, k], xctb_r[:, k, t0:t0 + csz])
            return xc

        # Startup: interleave chunk-0 activations with w1's first f-chunk
        # per k so the first matmul group is runnable after ~2MB of DMA.
        # Then w1 f-chunk-major with w2 k-slices interleaved at a ratio
        # that keeps DMA just ahead of PE's w1 consumption, so w2 is
        # resident before chunk-0 matmul2 starts (~70us in).
        w1_s = const.tile([P, KH, F], dt.bfloat16)
        w2_s = const.tile([P, KF, H], dt.bfloat16)
        xc0 = xpool.tile([P, KH, 512], dt.bfloat16, name="xc")[:, :, :chunks[0][1]]
        for k in range(KH):
            nc.sync.dma_start(xc0[:, k], xctb_r[:, k, 0:chunks[0][1]])
            nc.sync.dma_start(w1_s[:, k, 0:512], w1_r[:, k, 0:512])
        for k in range(4):
            nc.sync.dma_start(w2_s[:, k], w2_r[:, k])
        for fc in range(1, F // 512):
            for k in range(KH):
                nc.sync.dma_start(
                    w1_s[:, k, fc * 512:(fc + 1) * 512],
                    w1_r[:, k, fc * 512:(fc + 1) * 512])
            for k in range(4 * fc, 4 * fc + 4):
                nc.sync.dma_start(w2_s[:, k], w2_r[:, k])

        def emit_mm1(xc, csz):
            h1 = h1pool.tile([P, KF, 512], dt.bfloat16, name="h1")[:, :, :csz]
            for f in range(KF):
                ps1 = p1pool.tile([P, 512], dt.float32, name="ps1")[:, :csz]
                for k in range(KH):
                    nc.tensor.matmul(
                        ps1[:], w1_s[:, k, f * P:(f + 1) * P], xc[:, k],
                        start=(k == 0), stop=(k == KH - 1),
                    )
                nc.scalar.activation(h1[:, f], ps1[:], AF.Gelu, bias=b1_s[:, f:f + 1])
            return h1

        def emit_mm2(h1, t0, csz):
            for ct in range(csz // P):
                gt = t0 // P + ct
                for hh in range(H // 512):
                    ps2 = p2pool.tile([P, 512], dt.float32)
                    for k in range(KF):
                        nc.tensor.matmul(
                            ps2[:], h1[:, k, ct * P:(ct + 1) * P],
                            w2_s[:, k, hh * 512:(hh + 1) * 512],
                            start=(k == 0), stop=(k == KF - 1),
                        )
                    ob = opool.tile([P, 512], dt.float32)
                    nc.vector.tensor_tensor(
                        ob[:], ps2[:], b2b_s[:, hh * 512:(hh + 1) * 512], ALU.add)
                    nc.vector.tensor_scalar_mul(ob[:], ob[:], wmat[:, gt:gt + 1])
                    nc.sync.dma_start(
                        yc[gt * P:(gt + 1) * P, hh * 512:(hh + 1) * 512], ob[:])

        # Chunk-0 first FFN matmul overlaps the router's DMAs.
        h1_0 = emit_mm1(xc0, chunks[0][1])

        # ---- Router: combine weight of MY expert for my gathered tokens ----
        for tt in range(TTc):
            xt_t = xrpool.tile([P, KH, P], dt.float32)
            nc.sync.dma_start(xt_t[:], xct32_r[:, :, tt * P:(tt + 1) * P])
            lg = rpsum.tile([P, E], dt.float32)
            for k in range(KH):
                nc.tensor.matmul(
                    lg[:], xt_t[:, k], rw_s[:, k],
                    start=(k == 0), stop=(k == KH - 1),
                )
            l = rpool.tile([P, E], dt.float32)
            nc.vector.tensor_tensor(l[:], lg[:], rbb_s[:], ALU.add)
            m1 = rpool.tile([P, 1], dt.float32)
            nc.vector.reduce_max(m1[:], l[:], axis=AX.X)
            nm1 = rpool.tile([P, 1], dt.float32)
            nc.vector.tensor_scalar_mul(nm1[:], m1[:], -1.0)
            ismax = rpool.tile([P, E], dt.float32)
            nc.vector.tensor_tensor(
                ismax[:], l[:], m1[:].to_broadcast((P, E)), ALU.is_equal)
            pen = rpool.tile([P, E], dt.float32)
            nc.vector.tensor_scalar_mul(pen[:], ismax[:], 1e30)
            lmask = rpool.tile([P, E], dt.float32)
            nc.vector.tensor_tensor(lmask[:], l[:], pen[:], ALU.subtract)
            m2 = rpool.tile([P, 1], dt.float32)
            nc.vector.reduce_max(m2[:], lmask[:], axis=AX.X)
            lsel = rpool.tile([P, E], dt.float32)
            nc.vector.tensor_tensor(lsel[:], l[:], selb_s[:], ALU.mult)
            lmine = rpool.tile([P, 1], dt.float32)
            nc.vector.reduce_sum(lmine[:], lsel[:], axis=AX.X)
            ge = rpool.tile([P, 1], dt.float32)
            nc.vector.tensor_tensor(ge[:], lmine[:], m2[:], ALU.is_ge)
            e1 = rpool.tile([P, 1], dt.float32)
            nc.scalar.activation(e1[:], lmine[:], AF.Exp, bias=nm1[:])
            e2 = rpool.tile([P, 1], dt.float32)
            nc.scalar.activation(e2[:], m2[:], AF.Exp, bias=nm1[:])
            den = rpool.tile([P, 1], dt.float32)
            nc.vector.tensor_scalar_add(den[:], e2[:], 1.0)
            rec = rpool.tile([P, 1], dt.float32)
            nc.vector.reciprocal(rec[:], den[:])
            wnum = rpool.tile([P, 1], dt.float32)
            nc.vector.tensor_tensor(wnum[:], e1[:], ge[:], ALU.mult)
            nc.vector.tensor_tensor(wmat[:, tt:tt + 1], wnum[:], rec[:], ALU.mult)

        # ---- Expert FFN over gathered tokens, weighted output ----
        emit_mm2(h1_0, chunks[0][0], chunks[0][1])
        for (t0, csz) in chunks[1:]:
            xc = load_xc(t0, csz)
            h1 = emit_mm1(xc, csz)
            emit_mm2(h1, t0, csz)
    return nc


def _get_nc(C):
    if C not in _CACHE:
        nc = _build_nc(C)
        nc.finalize()
        _CACHE[C] = nc
    return _CACHE[C]


def dispatch(hidden_states, router_w, router_b):
    """Host-side top-2 dispatch: per-expert token index lists + capacity."""
    x = np.asarray(hidden_states, dtype=np.float32).reshape(T, H)
    logits = x @ np.asarray(router_w, dtype=np.float32)
    logits = logits + np.asarray(router_b, dtype=np.float32)
    top2 = np.argpartition(logits, E - 2, axis=1)[:, E - 2:]  # [T, 2] unordered
    idx_lists = []
    for m in range(E):
        idx_lists.append(np.where((top2 == m).any(axis=1))[0])
    cmax = max(len(ix) for ix in idx_lists)
    C = max(P, ((cmax + P - 1) // P) * P)
    return x, idx_lists, C


def make_in_maps(hidden_states, router_w, router_b, w1, b1, w2, b2):
    bf16 = ml_dtypes.bfloat16
    x, idx_lists, C = dispatch(hidden_states, router_w, router_b)
    xt = np.ascontiguousarray(x.T)            # [H, T] f32
    xtb = xt.astype(bf16)
    rw = np.ascontiguousarray(np.asarray(router_w, dtype=np.float32))
    rbb = np.ascontiguousarray(
        np.broadcast_to(np.asarray(router_b, dtype=np.float32), (P, E)))
    w1 = np.asarray(w1, dtype=np.float32)
    w2 = np.asarray(w2, dtype=np.float32)
    b1 = np.asarray(b1, dtype=np.float32)
    b2 = np.asarray(b2, dtype=np.float32)
    in_maps = []
    for m in range(E):
        ix = idx_lists[m]
        pad = np.zeros(C, dtype=np.int64)
        pad[:len(ix)] = ix
        sel = np.zeros((P, E), dtype=np.float32)
        sel[:, m] = 1.0
        in_maps.append({
            "xct32": np.ascontiguousarray(xt[:, pad]),
            "xctb": np.ascontiguousarray(xtb[:, pad]),
            "rw": rw,
            "rbb": rbb,
            "selb": sel,
            "w1d": np.ascontiguousarray(w1[m].astype(bf16)),
            "w2d": np.ascontiguousarray(w2[m].astype(bf16)),
            "b1d": np.ascontiguousarray(b1[m].reshape(KF, P).T),
            "b2b": np.ascontiguousarray(np.broadcast_to(b2[m], (P, H))),
        })
    return in_maps, idx_lists, C


def run_device(in_maps, C):
    from concourse.bass_utils import run_bass_kernel_spmd

    nc = _get_nc(C)
    res = run_bass_kernel_spmd(nc, in_maps, core_ids=list(range(E)))
    return res.results


def kernel(hidden_states, router_w, router_b, w1, b1, w2, b2):
    in_maps, idx_lists, C = make_in_maps(
        hidden_states, router_w, router_b, w1, b1, w2, b2)
    results = run_device(in_maps, C)
    acc = np.zeros((T, H), dtype=np.float32)
    for m in range(E):
        ix = idx_lists[m]
        acc[ix] += np.asarray(results[m]["yc"], dtype=np.float32)[:len(ix)]
    return acc.reshape(B, S, H)


# revision 26
# speedup vs baseline: 39869.6665x; 1.0268x over previous
"""MoE layer (8 experts, top-2) on 8 TRN2 NeuronCores, expert-parallel.

Strategy (sparse dispatch, per the sharding hint):
  - Core m owns expert m (w1[m], w2[m], b1[m], b2[m]).
  - Host computes top-2 expert ids per token (fp32 router, dispatch only)
    and "all-to-all"s: each core receives only the tokens routed to its
    expert, gathered as X_c^T [H, C] (C = max expert load, rounded to 128).
  - On device, each core re-runs the router (fp32 matmul on PE) over its
    gathered tokens and derives ITS OWN expert's combine weight per token
    purely elementwise:
        w_e(t) = exp(l_e - m1) / (1 + exp(m2 - m1))  if l_e >= m2 else 0
    (equals softmax-top2-renormalize of the reference).
  - FFN in bf16 (f32 PSUM accumulate): h1 = gelu(x @ w1 + b1) in [F, C]
    layout; y = (h1^T @ w2 + b2) * w with tokens on partitions -> yc [C, H].
  - Host scatter-adds each core's weighted outputs back to token order.
"""

from contextlib import ExitStack

import ml_dtypes
import numpy as np

P = 128
B, S, H, F, E = 2, 2048, 1024, 4096, 8
T = B * S            # 4096 tokens
KH = H // P          # 8   k-subtiles over H
KF = F // P          # 32  k-subtiles over F

_CACHE = {}


def _chunks(C):
    out = []
    t0 = 0
    while t0 < C:
        size = min(512, C - t0)
        out.append((t0, size))
        t0 += size
    return out


def _build_nc(C):
    import concourse.mybir as mybir
    import concourse.tile as tile
    from concourse import bacc

    dt = mybir.dt
    AF = mybir.ActivationFunctionType
    ALU = mybir.AluOpType
    AX = mybir.AxisListType

    TTc = C // P  # token tiles

    nc = bacc.Bacc(
        "TRN2", target_bir_lowering=False, debug=False, num_devices=E)

    xct32 = nc.declare_dram_parameter("xct32", [H, C], dt.float32, isOutput=False)
    xctb = nc.declare_dram_parameter("xctb", [H, C], dt.bfloat16, isOutput=False)
    rw = nc.declare_dram_parameter("rw", [H, E], dt.float32, isOutput=False)
    rbb = nc.declare_dram_parameter("rbb", [P, E], dt.float32, isOutput=False)
    selb = nc.declare_dram_parameter("selb", [P, E], dt.float32, isOutput=False)
    w1d = nc.declare_dram_parameter("w1d", [H, F], dt.bfloat16, isOutput=False)
    w2d = nc.declare_dram_parameter("w2d", [F, H], dt.bfloat16, isOutput=False)
    b1d = nc.declare_dram_parameter("b1d", [P, KF], dt.float32, isOutput=False)
    b2b = nc.declare_dram_parameter("b2b", [P, H], dt.float32, isOutput=False)
    yc = nc.declare_dram_parameter("yc", [C, H], dt.float32, isOutput=True)

    xct32_r = xct32.rearrange("(k p) t -> p k t", p=P)
    xctb_r = xctb.rearrange("(k p) t -> p k t", p=P)
    rw_r = rw.rearrange("(k p) e -> p k e", p=P)
    w1_r = w1d.rearrange("(k p) f -> p k f", p=P)
    w2_r = w2d.rearrange("(k p) h -> p k h", p=P)

    with ExitStack() as ctx:
        tc = ctx.enter_context(tile.TileContext(nc))
        const = ctx.enter_context(tc.tile_pool(name="const", bufs=1))
        xrpool = ctx.enter_context(tc.tile_pool(name="xr", bufs=2))
        rpool = ctx.enter_context(tc.tile_pool(name="rtmp", bufs=3))
        rpsum = ctx.enter_context(tc.tile_pool(name="rpsum", bufs=2, space="PSUM"))
        xpool = ctx.enter_context(tc.tile_pool(name="xc", bufs=2))
        h1pool = ctx.enter_context(tc.tile_pool(name="h1", bufs=1))
        p1pool = ctx.enter_context(tc.tile_pool(name="p1", bufs=3, space="PSUM"))
        p2pool = ctx.enter_context(tc.tile_pool(name="p2", bufs=3, space="PSUM"))
        opool = ctx.enter_context(tc.tile_pool(name="ob", bufs=6))

        # Small constants first so nothing queues behind the weight stacks.
        # (b2b is 0.5MB and not needed until the first output stage ~70us in,
        # so it loads after the weight stream instead.)
        rbb_s = const.tile([P, E], dt.float32)
        nc.sync.dma_start(rbb_s[:], rbb[:])
        selb_s = const.tile([P, E], dt.float32)
        nc.sync.dma_start(selb_s[:], selb[:])
        b1_s = const.tile([P, KF], dt.float32)
        nc.sync.dma_start(b1_s[:], b1d[:])
        rw_s = const.tile([P, KH, E], dt.float32)
        b2b_s = const.tile([P, H], dt.float32)
        wmat = const.tile([P, TTc], dt.float32)

        chunks = _chunks(C)

        def load_xc(t0, csz):
            xc = xpool.tile([P, KH, 512], dt.bfloat16, name="xc")[:, :, :csz]
            for k in range(KH):
                nc.sync.dma_start(xc[:, k], xctb_r[:, k, t0:t0 + csz])
            return xc

        # Startup: interleave chunk-0 activations with w1's first f-chunk
        # per k so the first matmul group is runnable after ~2MB of DMA.
        # Then w1 f-chunk-major with w2 k-slices interleaved at a ratio
        # that keeps DMA just ahead of PE's w1 consumption, so w2 is
        # resident before chunk-0 matmul2 starts (~70us in).
        w1_s = const.tile([P, KH, F], dt.bfloat16)
        w2_s = const.tile([P, KF, H], dt.bfloat16)
        xc0 = xpool.tile([P, KH, 512], dt.bfloat16, name="xc")[:, :, :chunks[0][1]]
        for k in range(KH):
            nc.sync.dma_start(xc0[:, k], xctb_r[:, k, 0:chunks[0][1]])
            nc.sync.dma_start(w1_s[:, k, 0:512], w1_r[:, k, 0:512])
        w2_next = 0
        for fc in range(1, F // 512):
            for k in range(KH):
                nc.sync.dma_start(
                    w1_s[:, k, fc * 512:(fc + 1) * 512],
                    w1_r[:, k, fc * 512:(fc + 1) * 512])
            share = 0 if fc < 2 else (5 if fc < 7 else KF - w2_next)
            for k in range(w2_next, w2_next + share):
                nc.sync.dma_start(w2_s[:, k], w2_r[:, k])
            w2_next += share
            if fc == 4:
                nc.sync.dma_start(rw_s[:], rw_r)
        nc.sync.dma_start(b2b_s[:], b2b[:])

        def emit_mm1(xc, csz):
            h1 = h1pool.tile([P, KF, 512], dt.bfloat16, name="h1")[:, :, :csz]
            for f in range(KF):
                ps1 = p1pool.tile([P, 512], dt.float32, name="ps1")[:, :csz]
                for k in range(KH):
                    nc.tensor.matmul(
                        ps1[:], w1_s[:, k, f * P:(f + 1) * P], xc[:, k],
                        start=(k == 0), stop=(k == KH - 1),
                    )
                nc.scalar.activation(h1[:, f], ps1[:], AF.Gelu, bias=b1_s[:, f:f + 1])
            return h1

        def emit_mm2(h1, t0, csz):
            for ct in range(csz // P):
                gt = t0 // P + ct
                for hh in range(H // 512):
                    ps2 = p2pool.tile([P, 512], dt.float32)
                    for k in range(KF):
                        nc.tensor.matmul(
                            ps2[:], h1[:, k, ct * P:(ct + 1) * P],
                            w2_s[:, k, hh * 512:(hh + 1) * 512],
                            start=(k == 0), stop=(k == KF - 1),
                        )
                    ob = opool.tile([P, 512], dt.float32)
                    nc.vector.tensor_tensor(
                        ob[:], ps2[:], b2b_s[:, hh * 512:(hh + 1) * 512], ALU.add)
                    nc.vector.tensor_scalar_mul(ob[:], ob[:], wmat[:, gt:gt + 1])
                    nc.sync.dma_start(
                        yc[gt * P:(gt + 1) * P, hh * 512:(hh + 1) * 512], ob[:])

        # Chunk-0 first FFN matmul overlaps the router's DMAs.
        h1_0 = emit_mm1(xc0, chunks[0][1])

        # ---- Router: combine weight of MY expert for my gathered tokens ----
        for tt in range(TTc):
            xt_t = xrpool.tile([P, KH, P], dt.float32)
            nc.sync.dma_start(xt_t[:], xct32_r[:, :, tt * P:(tt + 1) * P])
            lg = rpsum.tile([P, E], dt.float32)
            for k in range(KH):
                nc.tensor.matmul(
                    lg[:], xt_t[:, k], rw_s[:, k],
                    start=(k == 0), stop=(k == KH - 1),
                )
            l = rpool.tile([P, E], dt.float32)
            nc.vector.tensor_tensor(l[:], lg[:], rbb_s[:], ALU.add)
            m1 = rpool.tile([P, 1], dt.float32)
            nc.vector.reduce_max(m1[:], l[:], axis=AX.X)
            nm1 = rpool.tile([P, 1], dt.float32)
            nc.vector.tensor_scalar_mul(nm1[:], m1[:], -1.0)
            ismax = rpool.tile([P, E], dt.float32)
            nc.vector.tensor_tensor(
                ismax[:], l[:], m1[:].to_broadcast((P, E)), ALU.is_equal)
            pen = rpool.tile([P, E], dt.float32)
            nc.vector.tensor_scalar_mul(pen[:], ismax[:], 1e30)
            lmask = rpool.tile([P, E], dt.float32)
            nc.vector.tensor_tensor(lmask[:], l[:], pen[:], ALU.subtract)
            m2 = rpool.tile([P, 1], dt.float32)
            nc.vector.reduce_max(m2[:], lmask[:], axis=AX.X)
            lsel = rpool.tile([P, E], dt.float32)
            nc.vector.tensor_tensor(lsel[:], l[:], selb_s[:], ALU.mult)
            lmine = rpool.tile([P, 1], dt.float32)
            nc.vector.reduce_sum(lmine[:], lsel[:], axis=AX.X)
            ge = rpool.tile([P, 1], dt.float32)
            nc.vector.tensor_tensor(ge[:], lmine[:], m2[:], ALU.is_ge)
            e1 = rpool.tile([P, 1], dt.float32)
            nc.scalar.activation(e1[:], lmine[:], AF.Exp, bias=nm1[:])
            e2 = rpool.tile([P, 1], dt.float32)
            nc.scalar.activation(e2[:], m2[:], AF.Exp, bias=nm1[:])
            den = rpool.tile([P, 1], dt.float32)
            nc.vector.tensor_scalar_add(den[:], e2[:], 1.0)
            rec = rpool.tile([P, 1], dt.float32)
            nc.vector.reciprocal(rec[:], den[:])
            wnum = rpool.tile([P, 1], dt.float32)
            nc.vector.tensor_tensor(wnum[:], e1[:], ge[:], ALU.mult)
            nc.vector.tensor_tensor(wmat[:, tt:tt + 1], wnum[:], rec[:], ALU.mult)

        # ---- Expert FFN over gathered tokens, weighted output ----
        emit_mm2(h1_0, chunks[0][0], chunks[0][1])
        for (t0, csz) in chunks[1:]:
            xc = load_xc(t0, csz)
            h1 = emit_mm1(xc, csz)
            emit_mm2(h1, t0, csz)
    return nc


def _get_nc(C):
    if C not in _CACHE:
        nc = _build_nc(C)
        nc.finalize()
        _CACHE[C] = nc
    return _CACHE[C]


def dispatch(hidden_states, router_w, router_b):
    """Host-side top-2 dispatch: per-expert token index lists + capacity."""
    x = np.asarray(hidden_states, dtype=np.float32).reshape(T, H)
    logits = x @ np.asarray(router_w, dtype=np.float32)
    logits = logits + np.asarray(router_b, dtype=np.float32)
    top2 = np.argpartition(logits, E - 2, axis=1)[:, E - 2:]  # [T, 2] unordered
    idx_lists = []
    for m in range(E):
        idx_lists.append(np.where((top2 == m).any(axis=1))[0])
    cmax = max(len(ix) for ix in idx_lists)
    C = max(P, ((cmax + P - 1) // P) * P)
    return x, idx_lists, C


def make_in_maps(hidden_states, router_w, router_b, w1, b1, w2, b2):
    bf16 = ml_dtypes.bfloat16
    x, idx_lists, C = dispatch(hidden_states, router_w, router_b)
    xt = np.ascontiguousarray(x.T)            # [H, T] f32
    xtb = xt.astype(bf16)
    rw = np.ascontiguousarray(np.asarray(router_w, dtype=np.float32))
    rbb = np.ascontiguousarray(
        np.broadcast_to(np.asarray(router_b, dtype=np.float32), (P, E)))
    w1 = np.asarray(w1, dtype=np.float32)
    w2 = np.asarray(w2, dtype=np.float32)
    b1 = np.asarray(b1, dtype=np.float32)
    b2 = np.asarray(b2, dtype=np.float32)
    in_maps = []
    for m in range(E):
        ix = idx_lists[m]
        pad = np.zeros(C, dtype=np.int64)
        pad[:len(ix)] = ix
        sel = np.zeros((P, E), dtype=np.float32)
        sel[:, m] = 1.0
        in_maps.append({
            "xct32": np.ascontiguousarray(xt[:, pad]),
            "xctb": np.ascontiguousarray(xtb[:, pad]),
            "rw": rw,
            "rbb": rbb,
            "selb": sel,
            "w1d": np.ascontiguousarray(w1[m].astype(bf16)),
            "w2d": np.ascontiguousarray(w2[m].astype(bf16)),
            "b1d": np.ascontiguousarray(b1[m].reshape(KF, P).T),
            "b2b": np.ascontiguousarray(np.broadcast_to(b2[m], (P, H))),
        })
    return in_maps, idx_lists, C


def run_device(in_maps, C):
    from concourse.bass_utils import run_bass_kernel_spmd

    nc = _get_nc(C)
    res = run_bass_kernel_spmd(nc, in_maps, core_ids=list(range(E)))
    return res.results


def kernel(hidden_states, router_w, router_b, w1, b1, w2, b2):
    in_maps, idx_lists, C = make_in_maps(
        hidden_states, router_w, router_b, w1, b1, w2, b2)
    # One retry guards against a rare transient execution glitch observed on
    # the very first load of a freshly compiled NEFF (garbage ~1e35 values);
    # a healthy output has absmax of a few units.
    last_err = None
    for attempt in range(3):
        try:
            results = run_device(in_maps, C)
        except Exception as e:  # transient NRT/axon failures observed
            last_err = e
            import time as _time
            _time.sleep(10)
            continue
        acc = np.zeros((T, H), dtype=np.float32)
        for m in range(E):
            ix = idx_lists[m]
            acc[ix] += np.asarray(results[m]["yc"], dtype=np.float32)[:len(ix)]
        if np.isfinite(acc).all() and np.abs(acc).max() < 1e4:
            return acc.reshape(B, S, H)
    if last_err is not None:
        raise last_err
    return acc.reshape(B, S, H)
